# revision 1
# baseline (speedup 1.0000x reference)
"""Multi-head attention block (QKV proj + SDPA + merge-scramble + fc +
residual + LayerNorm) on 8 Trainium2 NeuronCores.

Sharding: data-parallel over the flattened batch dim (b*n = 32 sequences),
4 sequences per core. Each core runs an identical Bass program on its shard.

Per-sequence math (t = d = e = 512, H = 8 heads, dk = dv = 64):
  Q = qf @ w_q.T ; K = kf @ w_k.T ; V = vf @ w_v.T
  S_h = (Q_h K_h^T) / 8 ;  A_h = softmax(S_h) ;  O_h = A_h V_h
  x = merge_heads(O)            # [t, e]
  x = x.T (the reference's transpose+view scramble; legal since t == e)
  y = LN(x @ w_fc.T + qf) * gamma + beta

On-chip layout strategy: compute Q^T/K^T ([e, t], head-major on partitions)
and V ([t, e]) so that S^T = K_h Q_h^T comes out with tk on partitions.
Softmax then needs no max-subtraction (|S/8| < ~7) and no partition
reduction: exp runs elementwise on ScalarE, and the softmax denominators
fall out of the A^T V matmul by augmenting V with a ones column. The AV
matmul produces O^T tiles which assemble into x^T directly; one PE-transpose
pass converts x^T -> x for the fc matmul (the scramble means fc contracts
over the *time* index, so x must have time on partitions).

Matmuls run in float32r (TF32-ish split mode, 1 cycle/row at N>=512,
~1.5e-4 rel err) with fp32 PSUM accumulation.
"""

import numpy as np

import concourse.bacc as bacc
import concourse.mybir as mybir
import concourse.tile as tile
from concourse.bass_utils import run_bass_kernel_spmd
from concourse.masks import make_identity

F32 = mybir.dt.float32
F32R = mybir.dt.float32r
BF16 = mybir.dt.bfloat16
AF = mybir.ActivationFunctionType
OP = mybir.AluOpType

N_CORES = 8
S = 4          # sequences per core
T = 512        # sequence length
D = 512        # model dim (= e = n_head * d_k)
NH = 8         # heads
DV = 64        # head dim
C = 4          # 128-row chunks per 512 dim
P = 128
EPS = 1e-6

_PROGRAM_CACHE = {}


def _build_program(apply_affine: bool, loop_iters: int = 1):
    nc = bacc.Bacc()

    qT = nc.declare_dram_parameter("qT", [S, D, T], F32R, isOutput=False)
    kT = nc.declare_dram_parameter("kT", [S, D, T], F32R, isOutput=False)
    vT = nc.declare_dram_parameter("vT", [S, D, T], F32R, isOutput=False)
    qn = nc.declare_dram_parameter("qn", [S, T, D], F32, isOutput=False)
    wq = nc.declare_dram_parameter("wq", [D, D], F32R, isOutput=False)  # w_q.T
    wk = nc.declare_dram_parameter("wk", [D, D], F32R, isOutput=False)  # w_k.T
    wv = nc.declare_dram_parameter("wv", [D, D], F32R, isOutput=False)  # w_v.T
    wfc = nc.declare_dram_parameter("wfc", [D, D], F32R, isOutput=False)  # w_fc.T
    if apply_affine:
        gmb = nc.declare_dram_parameter("gmb", [P, D], F32, isOutput=False)
        btb = nc.declare_dram_parameter("btb", [P, D], F32, isOutput=False)
    out = nc.declare_dram_parameter("out", [S, T, D], F32, isOutput=True)

    with tile.TileContext(nc) as tc:
        with (
            tc.tile_pool(name="const", bufs=1) as cst,
            tc.tile_pool(name="inp", bufs=2) as inp,
            tc.tile_pool(name="proj", bufs=2) as proj,
            tc.tile_pool(name="expp", bufs=5) as expp,
            tc.tile_pool(name="xp", bufs=2) as xp,
            tc.tile_pool(name="small", bufs=2) as small,
            tc.tile_pool(name="psc", bufs=2, space="PSUM") as psc,
            tc.tile_pool(name="pfc", bufs=2, space="PSUM") as pfc,
            tc.tile_pool(name="pav", bufs=2, space="PSUM") as pavp,
            tc.tile_pool(name="ptr", bufs=2, space="PSUM") as ptrp,
        ):
            # one-time constants; weight DMAs split per 128-row chunk so the
            # first projection matmuls start as soon as chunk 0 lands.
            wq_sb = cst.tile([P, C, D], F32R, tag="wq")
            wk_sb = cst.tile([P, C, D], F32R, tag="wk")
            wv_sb = cst.tile([P, C, D], F32R, tag="wv")
            wfc_sb = cst.tile([P, C, D], F32R, tag="wfc")
            ident = cst.tile([P, P], F32, tag="ident")
            make_identity(nc, ident[:])
            eps_sb = cst.tile([P, 1], F32, tag="eps")
            nc.vector.memset(eps_sb[:], EPS)
            if apply_affine:
                gm_sb = cst.tile([P, D], F32, tag="gmb")
                bt_sb = cst.tile([P, D], F32, tag="btb")
                nc.sync.dma_start(gm_sb[:], gmb[:])
                nc.sync.dma_start(bt_sb[:], btb[:])

            def load(s, weight_dmas=None):
                st = {}
                st["qT"] = inp.tile([P, C, T], F32R, tag="qT", name="qT_sb")
                st["kT"] = inp.tile([P, C, T], F32R, tag="kT", name="kT_sb")
                st["vT"] = inp.tile([P, C, T], F32R, tag="vT", name="vT_sb")
                # consumption order: (wq,qT) all chunks, then (wk,kT), (wv,vT)
                for (sb, dr), w_pair in zip(
                    ((st["qT"], qT), (st["kT"], kT), (st["vT"], vT)),
                    weight_dmas or ((), (), ()),
                ):
                    for dc in range(C):
                        for w_sb, w in w_pair:
                            nc.sync.dma_start(
                                w_sb[:, dc, :],
                                w.rearrange("(c p) e -> p c e", p=P)[:, dc, :],
                            )
                        nc.sync.dma_start(
                            sb[:, dc, :],
                            dr[s].rearrange("(c p) t -> p c t", p=P)[:, dc, :],
                        )
                return st

            def projA(s, st):
                # Q^T/K^T [e, t] head-major; V [t, e] with per-head ones col
                st["QT"] = proj.tile([P, C, T], F32R, tag="QT", name="QT_sb")
                st["KT"] = proj.tile([P, C, T], F32R, tag="KT", name="KT_sb")
                for dst, w_sb, x_sb in (
                    (st["QT"], wq_sb, st["qT"]), (st["KT"], wk_sb, st["kT"])
                ):
                    for ec in range(C):
                        ps = pfc.tile([P, T], F32, tag="fc", name="ps")
                        for dc in range(C):
                            nc.tensor.matmul(
                                ps[:],
                                lhsT=w_sb[:, dc, ec * P:(ec + 1) * P],
                                rhs=x_sb[:, dc, :],
                                start=(dc == 0),
                                stop=(dc == C - 1),
                            )
                        nc.vector.tensor_copy(dst[:, ec, :], ps[:])
                V_sb = proj.tile([P, C, NH, DV + 1], BF16, tag="V", name="V_sb")
                st["V"] = V_sb
                nc.gpsimd.memset(V_sb[:, :, :, DV:DV + 1], 1.0)
                for tc_ in range(C):
                    ps = pfc.tile([P, T], F32, tag="fc", name="ps")
                    for dc in range(C):
                        nc.tensor.matmul(
                            ps[:],
                            lhsT=st["vT"][:, dc, tc_ * P:(tc_ + 1) * P],
                            rhs=wv_sb[:, dc, :],
                            start=(dc == 0),
                            stop=(dc == C - 1),
                        )
                    nc.scalar.copy(
                        V_sb[:, tc_, :, 0:DV],
                        ps.rearrange("p (h v) -> p h v", h=NH),
                    )

            def attnB(s, st):
                # S^T = K_h Q_h^T / 8 with tk on partitions -> exp elementwise
                # (no max subtraction; |S/8| <~ 7) -> O^T = V_aug^T A^T, whose
                # ones row yields the softmax denominators for free. Heads are
                # paired: rows 0-63/64-127 of a KT/QT chunk are disjoint PE
                # row groups, so back-to-back K=64 matmuls run concurrently.
                xT_sb = xp.tile([P, C, T], F32, tag="xT", name="xT_sb")
                sA = xp.tile([P, T], F32, tag="sA", name="sA")
                sB = xp.tile([P, T], F32, tag="sB", name="sB")
                st["xT"], st["sA"], st["sB"] = xT_sb, sA, sB
                nc.gpsimd.memset(sA[:], 1.0)
                nc.gpsimd.memset(sB[:], 1.0)
                for hp in range(NH // 2):
                    expSs = [
                        expp.tile([P, C, T], BF16, tag="expS", name=f"expS{sub}")
                        for sub in range(2)
                    ]
                    for tkc in range(C):
                        pss = []
                        for sub in range(2):
                            ps = psc.tile([P, T], F32, tag="sc", name="ps")
                            nc.tensor.matmul(
                                ps[:],
                                lhsT=st["KT"][sub * DV:(sub + 1) * DV, hp,
                                              tkc * P:(tkc + 1) * P],
                                rhs=st["QT"][sub * DV:(sub + 1) * DV, hp, :],
                                start=True,
                                stop=True,
                            )
                            pss.append(ps)
                        for sub in range(2):
                            nc.scalar.activation(
                                expSs[sub][:, tkc, :], pss[sub][:], AF.Exp,
                                scale=0.125,
                            )
                    for sub in range(2):
                        h = 2 * hp + sub
                        pav = pavp.tile([DV + 1, T], F32, tag="av", name="pav")
                        for tkc in range(C):
                            nc.tensor.matmul(
                                pav[:],
                                lhsT=st["V"][:, tkc, h, :],
                                rhs=expSs[sub][:, tkc, :],
                                start=(tkc == 0),
                                stop=(tkc == C - 1),
                            )
                        nc.vector.tensor_copy(
                            xT_sb[sub * DV:(sub + 1) * DV, hp, :], pav[0:DV, :]
                        )
                        s_t = sA if h < 4 else sB
                        nc.vector.tensor_copy(
                            s_t[32 * (h % 4):32 * (h % 4) + 1, :],
                            pav[DV:DV + 1, :],
                        )

            def tailC(s, st):
                # prefetch the residual rows early
                qn_cs = []
                for ac in range(C):
                    qn_c = small.tile([P, D], F32, tag="qn", bufs=4, name="qn_c")
                    nc.sync.dma_start(qn_c[:], qn[s, ac * P:(ac + 1) * P, :])
                    qn_cs.append(qn_c)
                st2_seq = small.tile([P, C, 2], F32, tag="st2", name="st2_seq")
                y_cs = []

                # R = 1/softmax-sums transposed to [tq, head]: sA/sB rows
                # {0,32,64,96} hold the sums; PE-transpose 128-col blocks and
                # take reciprocals of columns {0,32,64,96}.
                R_sb = small.tile([P, C, NH], F32, tag="R", name="R_sb")
                for c in range(C):
                    trS = ptrp.tile([P, T], F32, tag="tr", name="trS")
                    for i, s_t in enumerate((st["sA"], st["sB"])):
                        nc.tensor.transpose(
                            trS[:, i * P:(i + 1) * P],
                            s_t[:, c * P:(c + 1) * P],
                            ident[:],
                        )
                    nc.vector.reciprocal(R_sb[:, c, 0:4], trS[:, 0:97:32])
                    nc.vector.reciprocal(R_sb[:, c, 4:8], trS[:, P:P + 97:32])

                # x^T -> x (PE transpose) fused with softmax normalization
                x_nat = xp.tile([P, C, T], F32R, tag="xnat", name="x_nat")
                for c in range(C):
                    ptr = ptrp.tile([P, T], F32, tag="tr", name="ptr")
                    for ec in range(C):
                        nc.tensor.transpose(
                            ptr[:, ec * P:(ec + 1) * P],
                            st["xT"][:, ec, c * P:(c + 1) * P],
                            ident[:],
                        )
                    nc.vector.tensor_tensor(
                        x_nat[:, c, :].rearrange("p (h v) -> p h v", h=NH),
                        ptr.rearrange("p (h v) -> p h v", h=NH),
                        R_sb[:, c, :, None].to_broadcast((P, NH, DV)),
                        OP.mult,
                    )

                # fc (contracting over the *time* index, thanks to the
                # reference's transpose-view scramble) + residual + LayerNorm
                for ac in range(C):
                    psy = pfc.tile([P, T], F32, tag="fc", name="psy")
                    for cc in range(C):
                        nc.tensor.matmul(
                            psy[:],
                            lhsT=x_nat[:, cc, ac * P:(ac + 1) * P],
                            rhs=wfc_sb[:, cc, :],
                            start=(cc == 0),
                            stop=(cc == C - 1),
                        )
                    y_c = small.tile([P, D], F32, tag="y", bufs=4, name="y_c")
                    nc.vector.tensor_tensor(y_c[:], psy[:], qn_cs[ac][:], OP.add)
                    st6 = small.tile([P, 6], F32, tag="st6", name="st6")
                    nc.vector.bn_stats(st6[:], y_c[:])
                    nc.vector.bn_aggr(st2_seq[:, ac, :], st6[:])
                    y_cs.append(y_c)
                sd = small.tile([P, C], F32, tag="sd", name="sd")
                rinv = small.tile([P, C], F32, tag="rinv", name="rinv")
                nc.scalar.activation(sd[:], st2_seq[:, :, 1], AF.Sqrt, bias=eps_sb[:])
                nc.vector.reciprocal(rinv[:], sd[:])
                for ac in range(C):
                    y_c = y_cs[ac]
                    nc.vector.tensor_scalar(
                        y_c[:], y_c[:], st2_seq[:, ac, 0:1], rinv[:, ac:ac + 1],
                        OP.subtract, OP.mult,
                    )
                    if apply_affine:
                        nc.vector.tensor_tensor(y_c[:], y_c[:], gm_sb[:], OP.mult)
                        nc.vector.tensor_tensor(y_c[:], y_c[:], bt_sb[:], OP.add)
                    nc.sync.dma_start(out[s, ac * P:(ac + 1) * P, :], y_c[:])

            # software-pipelined emission: proj of seq s+1 is emitted before
            # the tail of seq s so the scheduler can fill PE gaps in the
            # attention/normalize phases with next-sequence matmuls.
            def emit_all():
                sts = {}
                sts[0] = load(0, weight_dmas=(
                    ((wq_sb, wq),),
                    ((wk_sb, wk),),
                    ((wv_sb, wv), (wfc_sb, wfc)),
                ))
                projA(0, sts[0])
                sts[1] = load(1)
                attnB(0, sts[0])
                for s in range(1, S):
                    projA(s, sts[s])
                    if s + 1 < S:
                        sts[s + 1] = load(s + 1)
                    tailC(s - 1, sts[s - 1])
                    attnB(s, sts[s])
                tailC(S - 1, sts[S - 1])

            if loop_iters == 1:
                emit_all()
            else:
                with tc.For_i(0, loop_iters, 1):
                    emit_all()

    nc.finalize()
    return nc


def _get_program(apply_affine: bool, loop_iters: int = 1):
    key = (apply_affine, loop_iters)
    if key not in _PROGRAM_CACHE:
        _PROGRAM_CACHE[key] = _build_program(apply_affine, loop_iters)
    return _PROGRAM_CACHE[key]


def kernel(q, k, v, w_q, w_k, w_v, w_fc, ln_gamma, ln_beta, _res_holder=None):
    q = np.asarray(q, dtype=np.float32)
    k = np.asarray(k, dtype=np.float32)
    v = np.asarray(v, dtype=np.float32)
    w_q = np.asarray(w_q, dtype=np.float32)
    w_k = np.asarray(w_k, dtype=np.float32)
    w_v = np.asarray(w_v, dtype=np.float32)
    w_fc = np.asarray(w_fc, dtype=np.float32)
    ln_gamma = np.asarray(ln_gamma, dtype=np.float32)
    ln_beta = np.asarray(ln_beta, dtype=np.float32)

    b, n, t, d = q.shape
    B = b * n
    assert (b, n, t, d) == (8, 4, T, D), q.shape
    qf = q.reshape(B, t, d)
    kf = k.reshape(B, t, d)
    vf = v.reshape(B, t, d)

    apply_affine = not (
        np.all(ln_gamma == 1.0) and np.all(ln_beta == 0.0)
    )
    nc = _get_program(apply_affine)

    wq_t = np.ascontiguousarray(w_q.T)
    wk_t = np.ascontiguousarray(w_k.T)
    wv_t = np.ascontiguousarray(w_v.T)
    wfc_t = np.ascontiguousarray(w_fc.T)

    in_maps = []
    for c in range(N_CORES):
        sl = slice(S * c, S * (c + 1))
        m = {
            "qT": np.ascontiguousarray(qf[sl].transpose(0, 2, 1)),
            "kT": np.ascontiguousarray(kf[sl].transpose(0, 2, 1)),
            "vT": np.ascontiguousarray(vf[sl].transpose(0, 2, 1)),
            "qn": np.ascontiguousarray(qf[sl]),
            "wq": wq_t, "wk": wk_t, "wv": wv_t, "wfc": wfc_t,
        }
        if apply_affine:
            m["gmb"] = np.ascontiguousarray(
                np.broadcast_to(ln_gamma, (P, D)).astype(np.float32)
            )
            m["btb"] = np.ascontiguousarray(
                np.broadcast_to(ln_beta, (P, D)).astype(np.float32)
            )
        in_maps.append(m)

    res = run_bass_kernel_spmd(nc, in_maps, list(range(N_CORES)))
    if _res_holder is not None:
        _res_holder.append(res)
    full = np.concatenate([res.results[c]["out"] for c in range(N_CORES)], axis=0)
    return full.reshape(b, n, t, d).astype(np.float32)



# revision 2
# speedup vs baseline: 1.3780x; 1.3780x over previous
"""Multi-head attention block (QKV proj + SDPA + merge-scramble + fc +
residual + LayerNorm) on 8 Trainium2 NeuronCores.

Sharding: data-parallel over the flattened batch dim (b*n = 32 sequences),
4 sequences per core. Each core runs an identical Bass program on its shard.

Per-sequence math (t = d = e = 512, H = 8 heads, dk = dv = 64):
  Q = qf @ w_q.T ; K = kf @ w_k.T ; V = vf @ w_v.T
  S_h = (Q_h K_h^T) / 8 ;  A_h = softmax(S_h) ;  O_h = A_h V_h
  x = merge_heads(O)            # [t, e]
  x = x.T (the reference's transpose+view scramble; legal since t == e)
  y = LN(x @ w_fc.T + qf) * gamma + beta

On-chip layout strategy: compute Q^T/K^T ([e, t], head-major on partitions)
and V ([t, e], bf16, with a per-head ones column) so that S^T = K_h Q_h^T
comes out with tk on partitions. Softmax needs no max-subtraction
(|S/8| < ~7): exp runs elementwise on ScalarE into bf16 tiles expS.
The AV matmul runs in O-form: lhsT = 128x128 chunks of expS (bf16 -> FWL
fast weight loads), rhs = V_aug [tk, 65], so the output lands as natural-
layout x chunks [tq, v] in PSUM with the softmax denominator in the 65th
column -- a per-partition scalar. Normalization is then one [128,4]
strided reciprocal + one broadcast multiply per 4-head half-bank, and NO
PE transposes are needed anywhere: x goes straight into the fc matmul
(the reference's scramble makes fc contract over the time index, i.e.
lhsT = x natural with time on partitions).

Matmuls run in float32r (TF32-ish split mode, 1 cycle/row at N>=512,
~1.5e-4 rel err) with fp32 PSUM accumulation; S^T head pairs use disjoint
PE row groups (partitions 0-63 / 64-127) and run concurrently.
"""

import numpy as np

import concourse.bacc as bacc
import concourse.mybir as mybir
import concourse.tile as tile
from concourse.bass_utils import run_bass_kernel_spmd

F32 = mybir.dt.float32
F32R = mybir.dt.float32r
BF16 = mybir.dt.bfloat16
AF = mybir.ActivationFunctionType
OP = mybir.AluOpType

N_CORES = 8
S = 4          # sequences per core
T = 512        # sequence length
D = 512        # model dim (= e = n_head * d_k)
NH = 8         # heads
DV = 64        # head dim
C = 4          # 128-row chunks per 512 dim
P = 128
EPS = 1e-6

_PROGRAM_CACHE = {}


def _build_program(apply_affine: bool, loop_iters: int = 1):
    nc = bacc.Bacc()

    qT = nc.declare_dram_parameter("qT", [S, D, T], F32R, isOutput=False)
    kT = nc.declare_dram_parameter("kT", [S, D, T], F32R, isOutput=False)
    vT = nc.declare_dram_parameter("vT", [S, D, T], F32R, isOutput=False)
    qn = nc.declare_dram_parameter("qn", [S, T, D], F32, isOutput=False)
    wq = nc.declare_dram_parameter("wq", [D, D], F32R, isOutput=False)  # w_q.T
    wk = nc.declare_dram_parameter("wk", [D, D], F32R, isOutput=False)  # w_k.T
    wv = nc.declare_dram_parameter("wv", [D, D], F32R, isOutput=False)  # w_v.T
    wfc = nc.declare_dram_parameter("wfc", [D, D], F32R, isOutput=False)  # w_fc.T
    if apply_affine:
        gmb = nc.declare_dram_parameter("gmb", [P, D], F32, isOutput=False)
        btb = nc.declare_dram_parameter("btb", [P, D], F32, isOutput=False)
    out = nc.declare_dram_parameter("out", [S, T, D], F32, isOutput=True)

    with tile.TileContext(nc) as tc:
        with (
            tc.tile_pool(name="const", bufs=1) as cst,
            tc.tile_pool(name="inp", bufs=2) as inp,
            tc.tile_pool(name="proj", bufs=2) as proj,
            tc.tile_pool(name="expp", bufs=9) as expp,
            tc.tile_pool(name="xp", bufs=2) as xp,
            tc.tile_pool(name="small", bufs=2) as small,
            tc.tile_pool(name="psc", bufs=2, space="PSUM") as psc,
            tc.tile_pool(name="pfc", bufs=2, space="PSUM") as pfc,
            tc.tile_pool(name="pav", bufs=4, space="PSUM") as pavp,
        ):
            # one-time constants; weight DMAs split per 128-row chunk so the
            # first projection matmuls start as soon as chunk 0 lands.
            wq_sb = cst.tile([P, C, D], F32R, tag="wq")
            wk_sb = cst.tile([P, C, D], F32R, tag="wk")
            wv_sb = cst.tile([P, C, D], F32R, tag="wv")
            wfc_sb = cst.tile([P, C, D], F32R, tag="wfc")
            eps_sb = cst.tile([P, 1], F32, tag="eps")
            nc.vector.memset(eps_sb[:], EPS)
            if apply_affine:
                gm_sb = cst.tile([P, D], F32, tag="gmb")
                bt_sb = cst.tile([P, D], F32, tag="btb")
                nc.sync.dma_start(gm_sb[:], gmb[:])
                nc.sync.dma_start(bt_sb[:], btb[:])

            def load(s, weight_dmas=None):
                st = {}
                st["qT"] = inp.tile([P, C, T], F32R, tag="qT", name="qT_sb")
                st["kT"] = inp.tile([P, C, T], F32R, tag="kT", name="kT_sb")
                st["vT"] = inp.tile([P, C, T], F32R, tag="vT", name="vT_sb")
                # consumption order: (wq,qT) all chunks, then (wk,kT), (wv,vT)
                for (sb, dr), w_pair in zip(
                    ((st["qT"], qT), (st["kT"], kT), (st["vT"], vT)),
                    weight_dmas or ((), (), ()),
                ):
                    for dc in range(C):
                        for w_sb, w in w_pair:
                            nc.sync.dma_start(
                                w_sb[:, dc, :],
                                w.rearrange("(c p) e -> p c e", p=P)[:, dc, :],
                            )
                        nc.sync.dma_start(
                            sb[:, dc, :],
                            dr[s].rearrange("(c p) t -> p c t", p=P)[:, dc, :],
                        )
                return st

            def projA(s, st):
                # Q^T/K^T [e, t] head-major; V [t, e] with per-head ones col
                st["QT"] = proj.tile([P, C, T], F32R, tag="QT", name="QT_sb")
                st["KT"] = proj.tile([P, C, T], F32R, tag="KT", name="KT_sb")
                for dst, w_sb, x_sb in (
                    (st["QT"], wq_sb, st["qT"]), (st["KT"], wk_sb, st["kT"])
                ):
                    for ec in range(C):
                        ps = pfc.tile([P, T], F32, tag="fc", name="ps")
                        for dc in range(C):
                            nc.tensor.matmul(
                                ps[:],
                                lhsT=w_sb[:, dc, ec * P:(ec + 1) * P],
                                rhs=x_sb[:, dc, :],
                                start=(dc == 0),
                                stop=(dc == C - 1),
                            )
                        nc.vector.tensor_copy(dst[:, ec, :], ps[:])
                V_sb = proj.tile([P, C, NH, DV + 1], BF16, tag="V", name="V_sb")
                st["V"] = V_sb
                nc.gpsimd.memset(V_sb[:, :, :, DV:DV + 1], 1.0)
                for tc_ in range(C):
                    ps = pfc.tile([P, T], F32, tag="fc", name="ps")
                    for dc in range(C):
                        nc.tensor.matmul(
                            ps[:],
                            lhsT=st["vT"][:, dc, tc_ * P:(tc_ + 1) * P],
                            rhs=wv_sb[:, dc, :],
                            start=(dc == 0),
                            stop=(dc == C - 1),
                        )
                    nc.scalar.copy(
                        V_sb[:, tc_, :, 0:DV],
                        ps.rearrange("p (h v) -> p h v", h=NH),
                    )

            def attnB(s, st):
                # S^T = K_h Q_h^T / 8 with tk on partitions -> exp elementwise
                # (no max subtraction; |S/8| <~ 7) into bf16 expS tiles.
                # Heads are paired: rows 0-63/64-127 of a KT/QT chunk are
                # disjoint PE row groups, so back-to-back K=64 matmuls run
                # concurrently.
                st["expS"] = []
                for hp in range(NH // 2):
                    expSs = [
                        expp.tile([P, C, T], BF16, tag="expS", name=f"expS{sub}")
                        for sub in range(2)
                    ]
                    st["expS"] += expSs
                    for tkc in range(C):
                        pss = []
                        for sub in range(2):
                            ps = psc.tile([P, T], F32, tag="sc", name="ps")
                            nc.tensor.matmul(
                                ps[:],
                                lhsT=st["KT"][sub * DV:(sub + 1) * DV, hp,
                                              tkc * P:(tkc + 1) * P],
                                rhs=st["QT"][sub * DV:(sub + 1) * DV, hp, :],
                                start=True,
                                stop=True,
                            )
                            pss.append(ps)
                        for sub in range(2):
                            nc.scalar.activation(
                                expSs[sub][:, tkc, :], pss[sub][:], AF.Exp,
                                scale=0.125,
                            )

            def avC(s, st):
                # O-form AV: out[tq, v] = sum_tk A[tq, tk] V[tk, v] with the
                # expS chunk as the (FWL bf16) stationary operand. Four heads
                # share one PSUM bank (4 x 65 cols); col 64 of each head is
                # the softmax denominator (ones column of V_aug), normalized
                # away by a strided [128,4] reciprocal + broadcast multiply.
                x_sb = xp.tile([P, C, T], F32R, tag="x", name="x_sb")
                st["x"] = x_sb
                W = DV + 1
                for tqc in range(C):
                    pvs = [
                        pavp.tile([P, 4 * W], F32, tag="av", name=f"pav{i}")
                        for i in range(2)
                    ]
                    for h in range(NH):
                        pv = pvs[h // 4]
                        col = (h % 4) * W
                        for tkc in range(C):
                            nc.tensor.matmul(
                                pv[:, col:col + W],
                                lhsT=st["expS"][h][:, tkc, tqc * P:(tqc + 1) * P],
                                rhs=st["V"][:, tkc, h, :],
                                start=(tkc == 0),
                                stop=(tkc == C - 1),
                            )
                    for half in range(2):
                        pv = pvs[half]
                        rc = small.tile([P, 4], F32, tag="rc", bufs=4, name="rc")
                        nc.vector.reciprocal(rc[:], pv[:, DV:4 * W:W])
                        nc.vector.tensor_tensor(
                            x_sb[:, tqc, half * 256:(half + 1) * 256]
                                .rearrange("p (h v) -> p h v", h=4),
                            pv.rearrange("p (h x) -> p h x", h=4)[:, :, 0:DV],
                            rc[:, :, None].to_broadcast((P, 4, DV)),
                            OP.mult,
                        )

            def tailC(s, st):
                # prefetch the residual rows early
                qn_cs = []
                for ac in range(C):
                    qn_c = small.tile([P, D], F32, tag="qn", bufs=4, name="qn_c")
                    nc.sync.dma_start(qn_c[:], qn[s, ac * P:(ac + 1) * P, :])
                    qn_cs.append(qn_c)
                st2_seq = small.tile([P, C, 2], F32, tag="st2", name="st2_seq")
                y_cs = []

                # fc (contracting over the *time* index, thanks to the
                # reference's transpose-view scramble) + residual + LayerNorm
                for ac in range(C):
                    psy = pfc.tile([P, T], F32, tag="fc", name="psy")
                    for cc in range(C):
                        nc.tensor.matmul(
                            psy[:],
                            lhsT=st["x"][:, cc, ac * P:(ac + 1) * P],
                            rhs=wfc_sb[:, cc, :],
                            start=(cc == 0),
                            stop=(cc == C - 1),
                        )
                    y_c = small.tile([P, D], F32, tag="y", bufs=4, name="y_c")
                    nc.vector.tensor_tensor(y_c[:], psy[:], qn_cs[ac][:], OP.add)
                    st6 = small.tile([P, 6], F32, tag="st6", name="st6")
                    nc.vector.bn_stats(st6[:], y_c[:])
                    nc.vector.bn_aggr(st2_seq[:, ac, :], st6[:])
                    y_cs.append(y_c)
                sd = small.tile([P, C], F32, tag="sd", name="sd")
                rinv = small.tile([P, C], F32, tag="rinv", name="rinv")
                nc.scalar.activation(sd[:], st2_seq[:, :, 1], AF.Sqrt, bias=eps_sb[:])
                nc.vector.reciprocal(rinv[:], sd[:])
                for ac in range(C):
                    y_c = y_cs[ac]
                    nc.vector.tensor_scalar(
                        y_c[:], y_c[:], st2_seq[:, ac, 0:1], rinv[:, ac:ac + 1],
                        OP.subtract, OP.mult,
                    )
                    if apply_affine:
                        nc.vector.tensor_tensor(y_c[:], y_c[:], gm_sb[:], OP.mult)
                        nc.vector.tensor_tensor(y_c[:], y_c[:], bt_sb[:], OP.add)
                    nc.sync.dma_start(out[s, ac * P:(ac + 1) * P, :], y_c[:])

            # software-pipelined emission: proj of seq s+1 is emitted before
            # the AV/tail of seq s so the scheduler can fill PE gaps in the
            # attention/normalize phases with next-sequence matmuls.
            def emit_all():
                sts = {}
                sts[0] = load(0, weight_dmas=(
                    ((wq_sb, wq),),
                    ((wk_sb, wk),),
                    ((wv_sb, wv), (wfc_sb, wfc)),
                ))
                projA(0, sts[0])
                sts[1] = load(1)
                attnB(0, sts[0])
                for s in range(1, S):
                    projA(s, sts[s])
                    if s + 1 < S:
                        sts[s + 1] = load(s + 1)
                    avC(s - 1, sts[s - 1])
                    tailC(s - 1, sts[s - 1])
                    attnB(s, sts[s])
                avC(S - 1, sts[S - 1])
                tailC(S - 1, sts[S - 1])

            if loop_iters == 1:
                emit_all()
            else:
                with tc.For_i(0, loop_iters, 1):
                    emit_all()

    nc.finalize()
    return nc


def _get_program(apply_affine: bool, loop_iters: int = 1):
    key = (apply_affine, loop_iters)
    if key not in _PROGRAM_CACHE:
        _PROGRAM_CACHE[key] = _build_program(apply_affine, loop_iters)
    return _PROGRAM_CACHE[key]


def kernel(q, k, v, w_q, w_k, w_v, w_fc, ln_gamma, ln_beta, _res_holder=None):
    q = np.asarray(q, dtype=np.float32)
    k = np.asarray(k, dtype=np.float32)
    v = np.asarray(v, dtype=np.float32)
    w_q = np.asarray(w_q, dtype=np.float32)
    w_k = np.asarray(w_k, dtype=np.float32)
    w_v = np.asarray(w_v, dtype=np.float32)
    w_fc = np.asarray(w_fc, dtype=np.float32)
    ln_gamma = np.asarray(ln_gamma, dtype=np.float32)
    ln_beta = np.asarray(ln_beta, dtype=np.float32)

    b, n, t, d = q.shape
    B = b * n
    assert (b, n, t, d) == (8, 4, T, D), q.shape
    qf = q.reshape(B, t, d)
    kf = k.reshape(B, t, d)
    vf = v.reshape(B, t, d)

    apply_affine = not (
        np.all(ln_gamma == 1.0) and np.all(ln_beta == 0.0)
    )
    nc = _get_program(apply_affine)

    wq_t = np.ascontiguousarray(w_q.T)
    wk_t = np.ascontiguousarray(w_k.T)
    wv_t = np.ascontiguousarray(w_v.T)
    wfc_t = np.ascontiguousarray(w_fc.T)

    in_maps = []
    for c in range(N_CORES):
        sl = slice(S * c, S * (c + 1))
        m = {
            "qT": np.ascontiguousarray(qf[sl].transpose(0, 2, 1)),
            "kT": np.ascontiguousarray(kf[sl].transpose(0, 2, 1)),
            "vT": np.ascontiguousarray(vf[sl].transpose(0, 2, 1)),
            "qn": np.ascontiguousarray(qf[sl]),
            "wq": wq_t, "wk": wk_t, "wv": wv_t, "wfc": wfc_t,
        }
        if apply_affine:
            m["gmb"] = np.ascontiguousarray(
                np.broadcast_to(ln_gamma, (P, D)).astype(np.float32)
            )
            m["btb"] = np.ascontiguousarray(
                np.broadcast_to(ln_beta, (P, D)).astype(np.float32)
            )
        in_maps.append(m)

    res = run_bass_kernel_spmd(nc, in_maps, list(range(N_CORES)))
    if _res_holder is not None:
        _res_holder.append(res)
    full = np.concatenate([res.results[c]["out"] for c in range(N_CORES)], axis=0)
    return full.reshape(b, n, t, d).astype(np.float32)
